# revision 1
# baseline (speedup 1.0000x reference)
"""Bilinear pooling kernel for 8 Trainium2 NeuronCores (Bass/Tile).

Computes out[b,n,v,o] = sum_{d,e} node[b,n,d] * veh[b,v,e] * W[o, d*E+e] + bias[o]
for B=16, N=64, V=16, D=E=128, O=256.

Strategy: tensor-shard over the output dim O (32 channels per core, no
communication). Per core, two matmul stages:
  Stage A:  U[d, (b,v,o)] = sum_e W3[o,d,e] * veh[b,v,e]
            32 matmuls: lhsT = W3[o].T [e=128, d=128], rhs = vehT [e=128, (b,v)=256]
  Stage B:  out[b][n, (v,o)] = sum_d node[b,n,d] * U[d, b, v, o]
            16 matmuls: lhsT = nodeT_b [d=128, n=64], rhs = U_b [d=128, (v,o)=512]
The host concatenates the 8 per-core [B,N,V,32] outputs along the last
axis and adds the bias during the unshard.
"""

import os
import sys

import numpy as np

sys.path.insert(0, "/opt/trn_rl_repo")

B, N, V = 16, 64, 16
D = 128
E = 128
O = 256
NCORES = 8
OS = O // NCORES  # 32 output channels per core
VO = V * OS  # 512

# f32  : plain fp32 matmuls (exact, 4 cycles/row)
# f32r : fp32 data in reduced-precision streaming mode (~2x faster than f32)
# bf16 : inputs cast to bf16 on host, full-rate matmuls
MODE = os.environ.get("BILIN_MODE", "bf16")

_nc_cache = {}


def _build(mode):
    from contextlib import ExitStack

    import concourse.tile as tile
    from concourse import bacc, mybir

    f32 = mybir.dt.float32
    if mode == "bf16":
        mmdt = mybir.dt.bfloat16
    elif mode == "f32r":
        mmdt = mybir.dt.float32r
    else:
        mmdt = f32

    nc = bacc.Bacc("TRN2", target_bir_lowering=False)
    nodeT_d = nc.dram_tensor("nodeT", [D, B * N], mmdt, kind="ExternalInput")
    vehT_d = nc.dram_tensor("vehT", [E, B * V], mmdt, kind="ExternalInput")
    wt_d = nc.dram_tensor("wt", [E, OS * D], mmdt, kind="ExternalInput")
    # n-major output so one [64, 2, 512] SBUF tile flushes as one DMA
    out_d = nc.dram_tensor("out", [N, B, VO], f32, kind="ExternalOutput")

    WSPLIT = [
        (0, 3, "sync"), (3, 6, "scalar"), (9, 7, "scalar"),
        (16, 8, "sync"), (24, 8, "gpsimd"),
    ]

    with ExitStack() as ctx:
        tc = ctx.enter_context(tile.TileContext(nc))
        const = ctx.enter_context(tc.tile_pool(name="const", bufs=1))
        wpool = ctx.enter_context(tc.tile_pool(name="w", bufs=1))
        upool = ctx.enter_context(tc.tile_pool(name="u", bufs=1))
        psum = ctx.enter_context(tc.tile_pool(name="psum", bufs=4, space="PSUM"))
        outp = ctx.enter_context(tc.tile_pool(name="outp", bufs=8))

        # PE warmup: ~3.5us of dummy matmuls on zeroed SBUF during the
        # input-DMA wait flips the HAM clock gate to 2.4 GHz before the
        # real matmuls start (otherwise much of the kernel runs at 1.2 GHz)
        warm = const.tile([D, B * V], mmdt)
        nc.vector.memset(warm[:], 0)
        wps = psum.tile([D, 4, B, V], f32, tag="ps")
        for i in range(18):
            nc.tensor.matmul(
                wps[:, i % 4], warm[:, 0:D], warm[:], start=True, stop=True
            )

        # stage A needs vehT + the first W channels first: graduated
        # chunks spread over the DMA rings, ordered by consumption time
        vehT = const.tile([E, B * V], mmdt)
        nc.sync.dma_start(vehT[:], vehT_d[:])
        engs = {"sync": nc.sync, "scalar": nc.scalar, "gpsimd": nc.gpsimd}
        wts = []
        for k, (o0, no, en) in enumerate(WSPLIT):
            wt = wpool.tile([E, no * D], mmdt, tag=f"wt{k}")
            engs[en].dma_start(wt[:], wt_d[:, o0 * D : (o0 + no) * D])
            wts.append((o0, no, wt))
        nodeT_t = const.tile([D, B * N], mmdt)
        nc.gpsimd.dma_start(nodeT_t[:], nodeT_d[:])
        nodeT = nodeT_t[:]

        def wsel(o):
            for o0, no, wt in wts:
                if o0 <= o < o0 + no:
                    return wt[:, (o - o0) * D : (o - o0 + 1) * D]
            raise AssertionError(o)

        # U[d, o, b, v] staged in SBUF for stage B (o-major so the stage-A
        # PSUM evacuation is one contiguous copy per psum tile)
        U = upool.tile([D, OS, B, V], mmdt)

        # Stage A: 8 psum tiles of [128, 4, 256] (2 banks, 4 o-channels)
        for g in range(OS // 4):
            pa = psum.tile([D, 4, B, V], f32, tag="ps")
            for i in range(4):
                o = 4 * g + i
                nc.tensor.matmul(
                    pa[:, i], wsel(o), vehT[:], start=True, stop=True,
                )
            if g % 2 == 0:
                nc.vector.tensor_copy(U[:, 4 * g : 4 * g + 4, :, :], pa[:])
            else:
                nc.scalar.copy(U[:, 4 * g : 4 * g + 4, :, :], pa[:])

        # Stage B: psum tiles [64, 2, 512] (2 banks, 2 batches); bias is
        # added on the host during unshard
        for p in range(B // 2):
            b0, b1 = 2 * p, 2 * p + 1
            pb = psum.tile([N, 2, VO], f32, tag="ps")
            nc.tensor.matmul(
                pb[:, 0], nodeT[:, b0 * N : (b0 + 1) * N], U[:, :, b0, :],
                start=True, stop=True,
            )
            nc.tensor.matmul(
                pb[:, 1], nodeT[:, b1 * N : (b1 + 1) * N], U[:, :, b1, :],
                start=True, stop=True,
            )
            ob = outp.tile([N, 2, VO], f32)
            if p % 2 == 0:
                nc.vector.tensor_copy(ob[:], pb[:])
            else:
                nc.scalar.copy(ob[:], pb[:])
            deng = nc.sync if p % 2 == 0 else nc.scalar
            deng.dma_start(out_d[:, b0 : b0 + 2, :], ob[:])

    nc.compile()
    return nc


def _get_nc(mode):
    if mode not in _nc_cache:
        _nc_cache[mode] = _build(mode)
    return _nc_cache[mode]


def _prep_inputs(node_embed, veh_fea, W, b, mode):
    if mode == "bf16":
        import ml_dtypes

        def cast(x):
            return np.ascontiguousarray(x.astype(ml_dtypes.bfloat16))
    else:

        def cast(x):
            return np.ascontiguousarray(x.astype(np.float32))

    node_embed = np.asarray(node_embed, dtype=np.float32)
    veh_fea = np.asarray(veh_fea, dtype=np.float32)
    W = np.asarray(W, dtype=np.float32)
    b = np.asarray(b, dtype=np.float32)

    nodeT = cast(node_embed.transpose(2, 0, 1).reshape(D, B * N))
    vehT = cast(veh_fea.transpose(2, 0, 1).reshape(E, B * V))
    W3 = W.reshape(O, D, E)

    in_maps = []
    for c in range(NCORES):
        sel = slice(c * OS, (c + 1) * OS)
        wt = cast(W3[sel].transpose(2, 0, 1).reshape(E, OS * D))
        in_maps.append({"nodeT": nodeT, "vehT": vehT, "wt": wt})
    return in_maps


def run(node_embed, veh_fea, W, b, trace=False):
    from concourse.bass_utils import run_bass_kernel_spmd

    nc = _get_nc(MODE)
    in_maps = _prep_inputs(node_embed, veh_fea, W, b, MODE)
    res = run_bass_kernel_spmd(nc, in_maps, list(range(NCORES)), trace=trace)
    # per-core out is [N, B, (o,v)] -> [B,N,V,OS]; bias added here (host)
    outs = [
        r["out"].reshape(N, B, OS, V).transpose(1, 0, 3, 2) for r in res.results
    ]
    full = np.concatenate(outs, axis=3) + np.asarray(b, np.float32)
    full = np.ascontiguousarray(full, dtype=np.float32)
    return full, res


def kernel(node_embed, veh_fea, W, b):
    return run(node_embed, veh_fea, W, b)[0]



# revision 5
# speedup vs baseline: 1.2147x; 1.2147x over previous
"""Bilinear pooling kernel for 8 Trainium2 NeuronCores (Bass/Tile).

Computes out[b,n,v,o] = sum_{d,e} node[b,n,d] * veh[b,v,e] * W[o, d*E+e] + bias[o]
for B=16, N=64, V=16, D=E=128, O=256.

Strategy: tensor-shard over the output dim O (32 channels per core, no
communication). Per core, two matmul stages pipelined in two o-halves:
  Stage A:  U[d, (o,b,v)] = sum_e W3[o,d,e] * veh[b,v,e]
            per o: lhsT = W3[o].T [e=128, d=128], rhs = vehT [e=128, (b,v)=256]
            psum groups of 4 channels, evacuated to bf16 U in SBUF by
            vector+gpsimd (half-group each).
  Stage B:  out[b][n, (o,v)] = sum_d node[b,n,d] * U[d, o-half, b, v]
            per b: lhsT = nodeT_b [d=128, n=64], rhs = U_h[:, :, b, :] [128, 256]
            2 batches packed per psum tile (partitions 0-63 / 64-127 via PE
            column tiling), 4 batches per 1-bank psum tile; scalar evacuates
            to bf16, sync DMAs 128KB contiguous blocks to DRAM.
The host concatenates the 8 per-core outputs along O and adds the bias.
"""

import sys

import numpy as np

sys.path.insert(0, "/opt/trn_rl_repo")

B, N, V = 16, 64, 16
D = 128
E = 128
O = 256
NCORES = 8
OS = O // NCORES  # 32 output channels per core

WARM = 8  # warmup matmuls to spin up the PE clock while inputs stream in

_nc_cache = {}


def _build():
    from contextlib import ExitStack

    import concourse.tile as tile
    from concourse import bacc, mybir

    f32 = mybir.dt.float32
    bf16 = mybir.dt.bfloat16

    nc = bacc.Bacc("TRN2", target_bir_lowering=False)
    # in0 = [vehT | wt channels 0:4]  (critical path: stage A group 0)
    in0_d = nc.dram_tensor("in0", [E, B * V + 4 * D], bf16, kind="ExternalInput")
    wtg1_d = nc.dram_tensor("wtg1", [E, 4 * D], bf16, kind="ExternalInput")
    wtg23_d = nc.dram_tensor("wtg23", [E, 8 * D], bf16, kind="ExternalInput")
    wtg45_d = nc.dram_tensor("wtg45", [E, 8 * D], bf16, kind="ExternalInput")
    wtg67_d = nc.dram_tensor("wtg67", [E, 8 * D], bf16, kind="ExternalInput")
    nodeT_d = nc.dram_tensor("nodeT", [D, B * N], bf16, kind="ExternalInput")
    # out tiles: t = h*4+q covers batches 4q..4q+3 of o-half h;
    # [128, 512] per tile: partition p=(b%2)*64+n, free = (j=(b%4)//2, ch, v)
    out_d = nc.dram_tensor("out", [8, 128, 512], bf16, kind="ExternalOutput")

    with ExitStack() as ctx:
        tc = ctx.enter_context(tile.TileContext(nc))
        const = ctx.enter_context(tc.tile_pool(name="const", bufs=1))
        upool = ctx.enter_context(tc.tile_pool(name="u", bufs=2))
        psA = ctx.enter_context(tc.tile_pool(name="psA", bufs=3, space="PSUM"))
        psB = ctx.enter_context(tc.tile_pool(name="psB", bufs=2, space="PSUM"))
        outp = ctx.enter_context(tc.tile_pool(name="outp", bufs=4))

        # ---- input DMAs, issued first so transfers start ASAP ----
        # (scalar issues none: it is a PSUM-evacuation engine with vector)
        in0 = const.tile([E, B * V + 4 * D], bf16)
        nc.sync.dma_start(in0[:], in0_d[:])
        wtg1 = const.tile([E, 4 * D], bf16)
        nc.sync.dma_start(wtg1[:], wtg1_d[:])
        nodeT_t = const.tile([D, B * N], bf16)
        nc.gpsimd.dma_start(nodeT_t[:], nodeT_d[:])
        wtg23 = const.tile([E, 8 * D], bf16)
        nc.gpsimd.dma_start(wtg23[:], wtg23_d[:])
        wtg45 = const.tile([E, 8 * D], bf16)
        nc.sync.dma_start(wtg45[:], wtg45_d[:])
        wtg67 = const.tile([E, 8 * D], bf16)
        nc.sync.dma_start(wtg67[:], wtg67_d[:])
        nodeT = nodeT_t[:]
        vehT = in0[:, : B * V]

        def wsel(o):
            # lhsT for stage-A channel o (core-local 0..31)
            if o < 4:
                return in0[:, B * V + o * D : B * V + (o + 1) * D]
            for lo, hi, t in ((4, 8, wtg1), (8, 16, wtg23), (16, 24, wtg45), (24, 32, wtg67)):
                if lo <= o < hi:
                    return t[:, (o - lo) * D : (o - lo + 1) * D]
            raise AssertionError(o)

        # ---- PE warmup on a zeroed tile (vector memset is its first op) ----
        warm = const.tile([E, B * V], bf16)
        nc.vector.memset(warm[:], 0)
        wps = psA.tile([D, 4, B * V], f32, tag="pa")
        for i in range(WARM):
            nc.tensor.matmul(wps[:, i % 4], warm[:, 0:D], warm[:], start=True, stop=True)

        U = [
            upool.tile([D, 16, B, V], bf16, tag="U", name=f"U{h}") for h in range(2)
        ]

        def stageA(g):
            # 4 channels (o = 4g .. 4g+3) -> psum [128, 4, 256] -> U[h]
            pa = psA.tile([D, 4, B * V], f32, tag="pa")
            for i in range(4):
                nc.tensor.matmul(pa[:, i], wsel(4 * g + i), vehT, start=True, stop=True)
            h, gl = divmod(g, 4)
            dst = U[h]
            nc.vector.tensor_copy(dst[:, 4 * gl : 4 * gl + 2, :, :], pa[:, 0:2])
            nc.scalar.copy(dst[:, 4 * gl + 2 : 4 * gl + 4, :, :], pa[:, 2:4])

        def stageB(h, q):
            # batches 4q..4q+3 of o-half h -> psum [128, 2, 256] -> out tile
            pb = psB.tile([N * 2, 2, 256], f32, tag="pb")
            for j in range(2):
                for pbi in range(2):
                    b = 4 * q + 2 * j + pbi
                    nc.tensor.matmul(
                        pb[64 * pbi : 64 * (pbi + 1), j],
                        nodeT[:, b * N : (b + 1) * N],
                        U[h][:, :, b, :],
                        start=True,
                        stop=True,
                    )
            ob = outp.tile([128, 512], bf16)
            if q % 2 == 0:
                nc.vector.tensor_copy(ob[:], pb[:])
            else:
                nc.scalar.copy(ob[:], pb[:])
            deng = nc.sync if q % 2 == 0 else nc.gpsimd
            deng.dma_start(out_d[4 * h + q], ob[:])

        # pipeline: A(h0) g0..g3, A(h1) g4 (hides U-copy latency), B(h0),
        # A(h1) g5..g7, B(h1)
        for g in range(5):
            stageA(g)
        for q in range(4):
            stageB(0, q)
        for g in range(5, 8):
            stageA(g)
        for q in range(4):
            stageB(1, q)

    nc.compile()
    return nc


def _get_nc():
    if "nc" not in _nc_cache:
        _nc_cache["nc"] = _build()
    return _nc_cache["nc"]


def _prep_inputs(node_embed, veh_fea, W, b):
    import ml_dtypes

    def cast(x):
        return np.ascontiguousarray(x.astype(ml_dtypes.bfloat16))

    node_embed = np.asarray(node_embed, dtype=np.float32)
    veh_fea = np.asarray(veh_fea, dtype=np.float32)
    W = np.asarray(W, dtype=np.float32)

    nodeT = cast(node_embed.transpose(2, 0, 1).reshape(D, B * N))
    vehT = veh_fea.transpose(2, 0, 1).reshape(E, B * V)
    W3 = W.reshape(O, D, E)

    in_maps = []
    for c in range(NCORES):
        # [E, o_local, D] channel-major weights for this core's O-shard
        wtc = W3[c * OS : (c + 1) * OS].transpose(2, 0, 1).reshape(E, OS * D)
        in_maps.append(
            {
                "in0": cast(np.concatenate([vehT, wtc[:, 0 : 4 * D]], axis=1)),
                "wtg1": cast(wtc[:, 4 * D : 8 * D]),
                "wtg23": cast(wtc[:, 8 * D : 16 * D]),
                "wtg45": cast(wtc[:, 16 * D : 24 * D]),
                "wtg67": cast(wtc[:, 24 * D : 32 * D]),
                "nodeT": nodeT,
            }
        )
    return in_maps


def run(node_embed, veh_fea, W, b, trace=False):
    from concourse.bass_utils import run_bass_kernel_spmd

    nc = _get_nc()
    in_maps = _prep_inputs(node_embed, veh_fea, W, b)
    res = run_bass_kernel_spmd(nc, in_maps, list(range(NCORES)), trace=trace)
    outs = []
    for r in res.results:
        # [8, 128, 512] -> [h, q, pb, n, j, ch, v] -> [b, n, v, (h,ch)]
        arr = np.asarray(r["out"]).astype(np.float32)
        arr = arr.reshape(2, 4, 2, 64, 2, 16, 16)
        arr = arr.transpose(1, 4, 2, 3, 6, 0, 5).reshape(B, N, V, OS)
        outs.append(arr)
    full = np.concatenate(outs, axis=3) + np.asarray(b, np.float32)
    return np.ascontiguousarray(full, dtype=np.float32), res


def kernel(node_embed, veh_fea, W, b):
    return run(node_embed, veh_fea, W, b)[0]
